# revision 22
# baseline (speedup 1.0000x reference)
"""CondConv (per-sample routed 3x3 conv) on 8 Trainium2 NeuronCores.

Reference computation (all fp32):
    gap     = mean(x, axis=(2,3))                    [B, CIN]
    routing = sigmoid(gap @ W_att.T + b_att)         [B, E]
    ker     = einsum('be,eoihw->boihw', routing, convs)
    out[b]  = conv2d(x[b], ker[b], stride 1, pad 1)  [B, COUT, 56, 56]

Sharding (B=32, COUT=256 across 8 cores): 4 core-pairs; pair p owns
samples 8p..8p+7 (batch data-parallel), and within a pair each core
computes one half of COUT (128 channels).

Per-core program (SPMD), bf16 datapath, fp32 PSUM accumulation:
  - expert bank resident in ONE SBUF tile [128cin, E*2304] so the whole
    bank loads as 6 large DMAs (vs 16 small ones); DMA order is
    xp(0) -> bank -> xp(1) -> xp(2) so sample 0's GAP/routing overlaps
    the bank load and the first matmul fires as early as possible.
  - routing on ScalarE(GAP accum + sigmoid)/DVE/GPSIMD; TensorE queue
    stays pure conv.
  - kernel mix on DVE as 8 tensor_scalar mults (4x bf16 mode) + 7
    tensor_tensor adds (2x bf16 mode) ~15.6us/sample, well under PE's
    ~25us/sample -- STT (no fast mode) would be 20.4us and starve the
    pipeline during the prologue. Samples 0/1 mix in column groups so
    conv(0) starts after only the first group.
  - conv: per sample 2chunk*9shift*7tile accumulating bf16 matmuls
    (N=448) into 7 PSUM tiles drawn from an 8-buffer rotating pool;
    the last accumulation round interleaves drains (ScalarE/DVE
    ping-pong) right behind each tile's final matmul so the next
    sample's matmuls never wait on PSUM recycling.
  - output: drains collect into one [128, 3136] SBUF tile, stored with
    a single DMA per sample (last sample: per-tile DMAs to cut the
    epilogue tail).
"""

import numpy as np

B, CIN, H, W = 32, 256, 56, 56
COUT, KK, E = 256, 3, 8
HP, WP = H + 2, W + 2          # zero-padded input plane
PHW = HP * WP                  # 3364
NSH = KK * KK                  # 9 shifts
CHUNKS = 2                     # CIN = 2 * 128
MHALF = COUT // 2              # couts per core
ROWS_PER_TILE = 8              # output rows per matmul tile
NTILES = H // ROWS_PER_TILE    # 7
NFREE = ROWS_PER_TILE * W      # 448
NCORES = 8
SAMPLES_PER_CORE = B // (NCORES // 2)  # 8
KCOLS = NSH * 128              # 1152 kernel cols per chunk
KWID = CHUNKS * KCOLS          # 2304 kernel cols per sample

_cached = {}


def _build_program():
    import concourse.bacc as bacc
    import concourse.bass_isa as bass_isa
    import concourse.mybir as mybir
    from concourse.tile import TileContext

    f32 = mybir.dt.float32
    bf16 = mybir.dt.bfloat16
    Alu = mybir.AluOpType
    Act = mybir.ActivationFunctionType

    nc = bacc.Bacc(None, target_bir_lowering=False)

    S = SAMPLES_PER_CORE
    xpad_d = nc.declare_dram_parameter(
        "xpad", [S, CHUNKS, 128, PHW], bf16, isOutput=False)
    bank_d = nc.declare_dram_parameter(
        "bank", [CHUNKS, 3, 128, E, 3 * 128], bf16, isOutput=False)
    watbat_d = nc.declare_dram_parameter("watbat", [128, 3 * E], f32,
                                         isOutput=False)
    out_d = nc.declare_dram_parameter(
        "out", [S, MHALF, H, W], bf16, isOutput=True)

    with TileContext(nc) as tc:
        with (
            tc.tile_pool(name="resident", bufs=1) as res_pool,
            tc.tile_pool(name="xp", bufs=3) as xp_pool,
            tc.tile_pool(name="kt", bufs=3) as kt_pool,
            tc.tile_pool(name="mixt", bufs=1) as t_pool,
            tc.tile_pool(name="mixu", bufs=2) as u_pool,
            tc.tile_pool(name="small", bufs=3) as small_pool,
            tc.tile_pool(name="outsb", bufs=2) as out_pool,
            tc.tile_pool(name="cpsum", bufs=8, space="PSUM") as cps_pool,
        ):
            # ---- small resident tiles -------------------------------------
            watbat_sb = res_pool.tile([128, 3 * E], f32, name="watbat",
                                      tag="watbat")
            nc.scalar.dma_start(out=watbat_sb[:], in_=watbat_d[:])
            # warm the ScalarE activation tables (Copy+Sigmoid) at t~0 --
            # dep-free via a GpSimd memset -- so neither the routing sigmoid
            # nor the first drain pays the table load on the critical path
            warm = small_pool.tile([128, 1], f32, name="warm", tag="warm")
            nc.gpsimd.memset(warm[:], 0.0)
            nc.scalar.activation(out=warm[:], in_=warm[:], func=Act.Copy)
            nc.scalar.activation(out=warm[:], in_=warm[:], func=Act.Sigmoid)
            # broadcast routing weights: scal[:, 8*b+e] = r_be on every
            # partition
            scal_sb = res_pool.tile([128, S * E], f32, name="scal", tag="scal")
            # whole expert bank in one tile: col = e*2304 + c*1152 + s*128 + m
            bank_sb = res_pool.tile([128, E * KWID], bf16, name="bank",
                                    tag="bank")

            def emit_bank_dma():
                v = bank_sb.rearrange("p (e q) -> p e q", e=E)
                for c in range(CHUNKS):
                    for g in range(3):
                        a = c * KCOLS + g * 384
                        nc.sync.dma_start(out=v[:, :, a:a + 384],
                                          in_=bank_d[c, g])

            def emit_xp(b, split=1):
                xp = []
                for c in range(CHUNKS):
                    t = xp_pool.tile([128, PHW], bf16, name=f"xp{c}",
                                     tag=f"xp{c}")
                    if split == 1:
                        nc.sync.dma_start(out=t[:], in_=xpad_d[b, c])
                    else:
                        piece = PHW // split
                        for j in range(split):
                            sl = slice(j * piece, (j + 1) * piece)
                            nc.sync.dma_start(out=t[:, sl],
                                              in_=xpad_d[b, c, :, sl])
                    xp.append(t)
                return xp

            def emit_gap(b, xp, split=1):
                """GAP via ScalarE in-place Copy whose accum_out yields the
                per-partition row sums (bf16 in, f32 accumulate)."""
                gq = []
                for c in range(CHUNKS):
                    pieces = []
                    w = PHW // split
                    for j in range(split):
                        sl = slice(j * w, (j + 1) * w)
                        g = small_pool.tile([128, 1], f32, name=f"gh{c}_{j}",
                                            tag=f"gh{c}_{j}")
                        nc.scalar.activation(out=xp[c][:, sl],
                                             in_=xp[c][:, sl],
                                             func=Act.Copy, accum_out=g[:])
                        pieces.append(g)
                    gq.append(pieces)
                return gq

            def emit_routing(b, gq):
                """logits[e] = sum_cin gap[cin]*W_att[e,cin]/3136 + b_att[e]
                (1/3136 folded into watbat host-side; b_att/128 lives in
                watbat[:, 2E:3E] so the partition all-reduce sums it back to
                b_att -- no separate bias add); sigmoid -> scal_sb."""
                gsum = []
                for c in range(CHUNKS):
                    if len(gq[c]) == 1:
                        gsum.append(gq[c][0])
                    else:
                        g = small_pool.tile([128, 1], f32, name=f"gs{c}",
                                            tag=f"gs{c}")
                        nc.vector.tensor_add(out=g[:], in0=gq[c][0][:],
                                             in1=gq[c][1][:])
                        gsum.append(g)
                t0 = small_pool.tile([128, E], f32, name="t0", tag="t0")
                nc.vector.scalar_tensor_tensor(
                    out=t0[:], in0=watbat_sb[:, 0:E],
                    scalar=gsum[0][:, 0:1],
                    in1=watbat_sb[:, 2 * E:3 * E], op0=Alu.mult, op1=Alu.add)
                t1 = small_pool.tile([128, E], f32, name="t1", tag="t1")
                nc.vector.scalar_tensor_tensor(
                    out=t1[:], in0=watbat_sb[:, E:2 * E],
                    scalar=gsum[1][:, 0:1],
                    in1=t0[:], op0=Alu.mult, op1=Alu.add)
                red = small_pool.tile([128, E], f32, name="red", tag="red")
                nc.gpsimd.partition_all_reduce(red[:], t1[:], channels=128,
                                               reduce_op=bass_isa.ReduceOp.add)
                nc.scalar.activation(out=scal_sb[:, b * E:(b + 1) * E],
                                     in_=red[:], func=Act.Sigmoid)

            def emit_mix(b, groups, scalar_experts=0):
                """Mix sample b's kernel on DVE with fast bf16 modes:
                t_e = r_be * bank_e   (tensor_scalar, 4x bf16)
                kt  = tree-sum(t_e)   (tensor_tensor adds, 2x bf16)
                `groups` splits the 2304 free cols so early groups unblock
                conv while later bank columns may still be in DMA flight.
                scalar_experts>0 offloads the last N experts' scaled copies
                to ScalarE (activation Copy with per-partition scale) --
                used for samples 0/1 where mix latency gates the conv and
                ScalarE is otherwise idle.
                """
                kt = kt_pool.tile([128, KWID], bf16, name="kt", tag="kt")
                ts = [t_pool.tile([128, KWID], bf16, name=f"t{e}",
                                  tag=f"t{e}") for e in range(E)]
                u = [u_pool.tile([128, KWID], bf16, name=f"u{i}", tag=f"u{i}")
                     for i in range(6)]
                r = lambda e: scal_sb[:, b * E + e:b * E + e + 1]
                for (a, z) in groups:
                    for e in range(E):
                        if e >= E - scalar_experts:
                            nc.scalar.activation(
                                out=ts[e][:, a:z],
                                in_=bank_sb[:, e * KWID + a:e * KWID + z],
                                func=Act.Copy, scale=r(e))
                        else:
                            nc.vector.tensor_scalar_mul(
                                out=ts[e][:, a:z],
                                in0=bank_sb[:, e * KWID + a:e * KWID + z],
                                scalar1=r(e))
                    pairs = [(u[0], ts[0], ts[1]), (u[1], ts[2], ts[3]),
                             (u[2], ts[4], ts[5]), (u[3], ts[6], ts[7]),
                             (u[4], u[0], u[1]), (u[5], u[2], u[3]),
                             (kt, u[4], u[5])]
                    for (o, i0, i1) in pairs:
                        nc.vector.tensor_add(out=o[:, a:z], in0=i0[:, a:z],
                                             in1=i1[:, a:z])
                return kt

            def emit_conv(b, xp, kt, dense, last):
                """Conv for sample b: accumulate 2c*9shift into 7 PSUM tiles
                from the 8-buffer rotating pool.

                dense=True (samples 1+): tiles 0,2,4 run all 18 rounds first
                (stopping 10-20us before sample end, DVE-drained at leisure),
                then tiles 1,3,5,6 round-major with ScalarE drains at sample
                end. With the +7 bank rotation, every drain the NEXT sample
                needs within ~7us comes from a dense tile that stopped long
                ago, so PSUM recycling never stalls the PE regardless of how
                the Tile scheduler orders drains among mix work.

                dense=False (sample 0): plain round-major, all drains on
                ScalarE -- required because sample 0's kt is mixed in column
                groups and round-major consumes kt shift by shift."""
                cps = [cps_pool.tile([128, NFREE], f32, name="cps", tag="cps")
                       for _ in range(NTILES)]
                x3 = [xp[c].rearrange("p (r q) -> p r q", q=WP)
                      for c in range(CHUNKS)]
                o = out_pool.tile([128, H * W], bf16, name="osb", tag="osb")

                def mm(n, c, s, first, stop):
                    dh, dw = s // KK, s % KK
                    lhsT = kt[:, c * KCOLS + s * 128:c * KCOLS + (s + 1) * 128]
                    rhs = x3[c][:, n * ROWS_PER_TILE + dh:
                                n * ROWS_PER_TILE + dh + ROWS_PER_TILE,
                                dw:dw + W]
                    nc.tensor.matmul(cps[n][:], lhsT, rhs,
                                     start=first, stop=stop)

                def drain(n, on_dve):
                    dst = o[:, n * NFREE:(n + 1) * NFREE]
                    if on_dve:
                        nc.vector.tensor_copy(out=dst, in_=cps[n][:])
                    else:
                        nc.scalar.activation(out=dst, in_=cps[n][:],
                                             func=Act.Copy)
                    if last:
                        # per-tile stores so the epilogue tail is one small
                        # DMA behind the final drain, not one big transfer
                        nc.sync.dma_start(
                            out=out_d[b, :,
                                      n * ROWS_PER_TILE:(n + 1) * ROWS_PER_TILE,
                                      :],
                            in_=o[:, n * NFREE:(n + 1) * NFREE])

                rounds = [(c, s) for c in range(CHUNKS) for s in range(NSH)]
                if dense == "semi":
                    # sample 1: kt arrives chunk-by-chunk barely ahead of
                    # the conv, so consume chunk0 as long as possible (t0's
                    # chunk0 rounds, then the late tiles' chunk0 rounds)
                    # before any chunk1 matmul; PSUM needs stay covered by
                    # sample 0's ScalarE drains.
                    late = (1, 3, 5, 6)
                    for i, (c, s) in enumerate(rounds[:NSH]):
                        mm(0, c, s, first=(i == 0), stop=False)
                    for i, (c, s) in enumerate(rounds[:NSH]):
                        for n in late:
                            mm(n, c, s, first=(i == 0), stop=False)
                    for j, (c, s) in enumerate(rounds[NSH:]):
                        mm(0, c, s, first=False, stop=(j == NSH - 1))
                    drain(0, on_dve=True)
                    for j, (c, s) in enumerate(rounds[NSH:]):
                        for n in late:
                            mm(n, c, s, first=False, stop=(j == NSH - 1))
                            if j == NSH - 1:
                                drain(n, on_dve=False)
                    for n in (2, 4):
                        for i, (c, s) in enumerate(rounds):
                            mm(n, c, s, first=(i == 0), stop=(i == 17))
                        drain(n, on_dve=True)
                    nc.sync.dma_start(out=out_d[b], in_=o[:])
                    return
                if dense:
                    for n in (0, 2, 4):
                        for i, (c, s) in enumerate(rounds):
                            mm(n, c, s, first=(i == 0), stop=(i == 17))
                        drain(n, on_dve=True)
                    late = (1, 3, 5, 6)
                else:
                    late = tuple(range(NTILES))
                for i, (c, s) in enumerate(rounds):
                    for k, n in enumerate(late):
                        mm(n, c, s, first=(i == 0), stop=(i == 17))
                        if i == 17:
                            # last sample: alternate engines so the final
                            # drains finish ~2x sooner
                            drain(n, on_dve=(last and k % 2 == 1))
                if not last:
                    nc.sync.dma_start(out=out_d[b], in_=o[:])

            # ---- software-pipelined emission ------------------------------
            # Sync-queue FIFO order gives DMA priority: watbat, xp(0), bank,
            # xp(1), xp(2), then per-iteration xp prefetch 3 samples ahead
            # behind each sample's output store. routing(b) is emitted with
            # gap(b) -- a full iteration before mix(b) -- so on the DVE queue
            # mix(b+2) sits right behind conv(b)'s ping-pong drains with its
            # sigmoid dependency long resolved, and the drains run the moment
            # conv(b) ends (PSUM recycling never waits on mix work).
            xps, gqs, kts = {}, {}, {}
            xps[0] = emit_xp(0)
            gqs[0] = emit_gap(0, xps[0])
            emit_bank_dma()
            emit_routing(0, gqs[0])
            kts[0] = emit_mix(0, [(0, 384), (384, 1152), (1152, KWID)])
            xps[1] = emit_xp(1)
            gqs[1] = emit_gap(1, xps[1])
            emit_routing(1, gqs[1])
            kts[1] = emit_mix(1, [(0, 1152), (1152, KWID)])
            xps[2] = emit_xp(2)
            gqs[2] = emit_gap(2, xps[2])
            emit_routing(2, gqs[2])
            for b in range(S):
                emit_conv(b, xps.pop(b), kts.pop(b), dense=(b > 0),
                          last=(b == S - 1))
                if b + 2 < S:
                    kts[b + 2] = emit_mix(b + 2, [(0, KWID)])
                if b + 3 < S:
                    xps[b + 3] = emit_xp(b + 3)
                    gqs[b + 3] = emit_gap(b + 3, xps[b + 3])
                    emit_routing(b + 3, gqs.pop(b + 3))

    nc.compile()
    return nc


def _prep_core_inputs(x, convs, W_att, b_att):
    """Host-side shard/layout prep. Returns list of 8 per-core input dicts."""
    import ml_dtypes
    f32 = np.float32
    bf16 = ml_dtypes.bfloat16
    # padded input, cin split into 2 chunks of 128
    xpad = np.zeros((B, CHUNKS, 128, HP, WP), dtype=bf16)
    xpad[:, :, :, 1:H + 1, 1:W + 1] = np.ascontiguousarray(x, dtype=f32).reshape(
        B, CHUNKS, 128, H, W).astype(bf16)
    xpad = xpad.reshape(B, CHUNKS, 128, PHW)

    # bank[half][c, g, p, e, kw*128 + m] = convs[e, half*128+m, c*128+p, g, kw]
    cv = np.ascontiguousarray(convs, dtype=f32).reshape(
        E, 2, MHALF, CHUNKS, 128, KK, KK)
    bank_halves = [
        np.ascontiguousarray(cv[:, h].transpose(2, 4, 3, 0, 5, 1).reshape(
            CHUNKS, 3, 128, E, 3 * 128)).astype(bf16)
        for h in range(2)
    ]

    watt = (np.asarray(W_att, dtype=f32).T / f32(H * W)).astype(f32)  # [CIN, E]
    watbat = np.empty((128, 3 * E), dtype=f32)
    watbat[:, 0:E] = watt[:128]
    watbat[:, E:2 * E] = watt[128:]
    # b_att/128 on every partition: the routing partition all-reduce sums it
    # back to b_att, so no separate bias add is needed
    watbat[:, 2 * E:3 * E] = np.broadcast_to(
        np.asarray(b_att, dtype=f32) / f32(128), (128, E))

    in_maps = []
    for k in range(NCORES):
        pair, half = k // 2, k % 2
        sl = slice(pair * SAMPLES_PER_CORE, (pair + 1) * SAMPLES_PER_CORE)
        in_maps.append({
            "xpad": np.ascontiguousarray(xpad[sl]),
            "bank": bank_halves[half],
            "watbat": watbat,
        })
    return in_maps


def _assemble_output(results):
    out = np.empty((B, COUT, H, W), dtype=np.float32)
    for k in range(NCORES):
        pair, half = k // 2, k % 2
        sl = slice(pair * SAMPLES_PER_CORE, (pair + 1) * SAMPLES_PER_CORE)
        out[sl, half * MHALF:(half + 1) * MHALF] = np.asarray(
            results[k]["out"], dtype=np.float32)
    return out


def kernel(x, convs, W_att, b_att):
    from concourse.bass_utils import run_bass_kernel_spmd

    if "nc" not in _cached:
        _cached["nc"] = _build_program()
    in_maps = _prep_core_inputs(x, convs, W_att, b_att)
    res = run_bass_kernel_spmd(_cached["nc"], in_maps, core_ids=list(range(NCORES)))
    return _assemble_output(res.results)


# revision 23
# speedup vs baseline: 1.1991x; 1.1991x over previous
"""CondConv (per-sample routed 3x3 conv) on 8 Trainium2 NeuronCores.

Reference computation (all fp32):
    gap     = mean(x, axis=(2,3))                    [B, CIN]
    routing = sigmoid(gap @ W_att.T + b_att)         [B, E]
    ker     = einsum('be,eoihw->boihw', routing, convs)
    out[b]  = conv2d(x[b], ker[b], stride 1, pad 1)  [B, COUT, 56, 56]

Sharding (B=32, COUT=256 across 8 cores): 4 core-pairs; pair p owns
samples 8p..8p+7 (batch data-parallel), and within a pair each core
computes one half of COUT (128 channels).

Per-core program (SPMD), bf16 datapath, fp32 PSUM accumulation:
  - expert bank resident in ONE SBUF tile [128cin, E*2304] so the whole
    bank loads as 6 large DMAs (vs 16 small ones); DMA order is
    xp(0) -> bank -> xp(1) -> xp(2) so sample 0's GAP/routing overlaps
    the bank load and the first matmul fires as early as possible.
  - routing on ScalarE(GAP accum + sigmoid)/DVE/GPSIMD; TensorE queue
    stays pure conv.
  - kernel mix on DVE as 8 tensor_scalar mults (4x bf16 mode) + 7
    tensor_tensor adds (2x bf16 mode) ~15.6us/sample, well under PE's
    ~25us/sample -- STT (no fast mode) would be 20.4us and starve the
    pipeline during the prologue. Samples 0/1 mix in column groups so
    conv(0) starts after only the first group.
  - conv: per sample 2chunk*9shift*7tile accumulating bf16 matmuls
    (N=448) into 7 PSUM tiles drawn from an 8-buffer rotating pool;
    the last accumulation round interleaves drains (ScalarE/DVE
    ping-pong) right behind each tile's final matmul so the next
    sample's matmuls never wait on PSUM recycling.
  - output: drains collect into one [128, 3136] SBUF tile, stored with
    a single DMA per sample (last sample: per-tile DMAs to cut the
    epilogue tail).
"""

import numpy as np

B, CIN, H, W = 32, 256, 56, 56
COUT, KK, E = 256, 3, 8
HP, WP = H + 2, W + 2          # zero-padded input plane
PHW = HP * WP                  # 3364
NSH = KK * KK                  # 9 shifts
CHUNKS = 2                     # CIN = 2 * 128
MHALF = COUT // 2              # couts per core
ROWS_PER_TILE = 8              # output rows per matmul tile
NTILES = H // ROWS_PER_TILE    # 7
NFREE = ROWS_PER_TILE * W      # 448
NCORES = 8
SAMPLES_PER_CORE = B // (NCORES // 2)  # 8
KCOLS = NSH * 128              # 1152 kernel cols per chunk
KWID = CHUNKS * KCOLS          # 2304 kernel cols per sample

_cached = {}


def _build_program():
    import concourse.bacc as bacc
    import concourse.bass_isa as bass_isa
    import concourse.mybir as mybir
    from concourse.tile import TileContext

    f32 = mybir.dt.float32
    bf16 = mybir.dt.bfloat16
    Alu = mybir.AluOpType
    Act = mybir.ActivationFunctionType

    nc = bacc.Bacc(None, target_bir_lowering=False)

    S = SAMPLES_PER_CORE
    xpad_d = nc.declare_dram_parameter(
        "xpad", [S, CHUNKS, 128, PHW], bf16, isOutput=False)
    bank_d = nc.declare_dram_parameter(
        "bank", [CHUNKS, 3, 128, E, 3 * 128], bf16, isOutput=False)
    watbat_d = nc.declare_dram_parameter("watbat", [128, 3 * E], f32,
                                         isOutput=False)
    out_d = nc.declare_dram_parameter(
        "out", [S, MHALF, H, W], bf16, isOutput=True)

    with TileContext(nc) as tc:
        with (
            tc.tile_pool(name="resident", bufs=1) as res_pool,
            tc.tile_pool(name="xp", bufs=3) as xp_pool,
            tc.tile_pool(name="kt", bufs=3) as kt_pool,
            tc.tile_pool(name="mixt", bufs=1) as t_pool,
            tc.tile_pool(name="mixu", bufs=2) as u_pool,
            tc.tile_pool(name="small", bufs=3) as small_pool,
            tc.tile_pool(name="outsb", bufs=2) as out_pool,
            tc.tile_pool(name="cpsum", bufs=8, space="PSUM") as cps_pool,
        ):
            # ---- small resident tiles -------------------------------------
            watbat_sb = res_pool.tile([128, 3 * E], f32, name="watbat",
                                      tag="watbat")
            nc.scalar.dma_start(out=watbat_sb[:], in_=watbat_d[:])
            # warm the ScalarE activation tables (Copy+Sigmoid) at t~0 --
            # dep-free via a GpSimd memset -- so neither the routing sigmoid
            # nor the first drain pays the table load on the critical path
            warm = small_pool.tile([128, 1], f32, name="warm", tag="warm")
            nc.gpsimd.memset(warm[:], 0.0)
            nc.scalar.activation(out=warm[:], in_=warm[:], func=Act.Copy)
            nc.scalar.activation(out=warm[:], in_=warm[:], func=Act.Sigmoid)
            # broadcast routing weights: scal[:, 8*b+e] = r_be on every
            # partition
            scal_sb = res_pool.tile([128, S * E], f32, name="scal", tag="scal")
            # whole expert bank in one tile: col = e*2304 + c*1152 + s*128 + m
            bank_sb = res_pool.tile([128, E * KWID], bf16, name="bank",
                                    tag="bank")

            def emit_bank_dma():
                v = bank_sb.rearrange("p (e q) -> p e q", e=E)
                for c in range(CHUNKS):
                    for g in range(3):
                        a = c * KCOLS + g * 384
                        nc.sync.dma_start(out=v[:, :, a:a + 384],
                                          in_=bank_d[c, g])

            def emit_xp(b, split=1):
                xp = []
                for c in range(CHUNKS):
                    t = xp_pool.tile([128, PHW], bf16, name=f"xp{c}",
                                     tag=f"xp{c}")
                    if split == 1:
                        nc.sync.dma_start(out=t[:], in_=xpad_d[b, c])
                    else:
                        piece = PHW // split
                        for j in range(split):
                            sl = slice(j * piece, (j + 1) * piece)
                            nc.sync.dma_start(out=t[:, sl],
                                              in_=xpad_d[b, c, :, sl])
                    xp.append(t)
                return xp

            def emit_gap(b, xp, split=1):
                """GAP via ScalarE in-place Copy whose accum_out yields the
                per-partition row sums (bf16 in, f32 accumulate)."""
                gq = []
                for c in range(CHUNKS):
                    pieces = []
                    w = PHW // split
                    for j in range(split):
                        sl = slice(j * w, (j + 1) * w)
                        g = small_pool.tile([128, 1], f32, name=f"gh{c}_{j}",
                                            tag=f"gh{c}_{j}")
                        nc.scalar.activation(out=xp[c][:, sl],
                                             in_=xp[c][:, sl],
                                             func=Act.Copy, accum_out=g[:])
                        pieces.append(g)
                    gq.append(pieces)
                return gq

            def emit_routing(b, gq):
                """logits[e] = sum_cin gap[cin]*W_att[e,cin]/3136 + b_att[e]
                (1/3136 folded into watbat host-side; b_att/128 lives in
                watbat[:, 2E:3E] so the partition all-reduce sums it back to
                b_att -- no separate bias add); sigmoid -> scal_sb."""
                gsum = []
                for c in range(CHUNKS):
                    if len(gq[c]) == 1:
                        gsum.append(gq[c][0])
                    else:
                        g = small_pool.tile([128, 1], f32, name=f"gs{c}",
                                            tag=f"gs{c}")
                        nc.vector.tensor_add(out=g[:], in0=gq[c][0][:],
                                             in1=gq[c][1][:])
                        gsum.append(g)
                t0 = small_pool.tile([128, E], f32, name="t0", tag="t0")
                nc.vector.scalar_tensor_tensor(
                    out=t0[:], in0=watbat_sb[:, 0:E],
                    scalar=gsum[0][:, 0:1],
                    in1=watbat_sb[:, 2 * E:3 * E], op0=Alu.mult, op1=Alu.add)
                t1 = small_pool.tile([128, E], f32, name="t1", tag="t1")
                nc.vector.scalar_tensor_tensor(
                    out=t1[:], in0=watbat_sb[:, E:2 * E],
                    scalar=gsum[1][:, 0:1],
                    in1=t0[:], op0=Alu.mult, op1=Alu.add)
                red = small_pool.tile([128, E], f32, name="red", tag="red")
                nc.gpsimd.partition_all_reduce(red[:], t1[:], channels=128,
                                               reduce_op=bass_isa.ReduceOp.add)
                nc.scalar.activation(out=scal_sb[:, b * E:(b + 1) * E],
                                     in_=red[:], func=Act.Sigmoid)

            def emit_mix(b, groups, scalar_experts=0):
                """Mix sample b's kernel on DVE with fast bf16 modes:
                t_e = r_be * bank_e   (tensor_scalar, 4x bf16)
                kt  = tree-sum(t_e)   (tensor_tensor adds, 2x bf16)
                `groups` splits the 2304 free cols so early groups unblock
                conv while later bank columns may still be in DMA flight.
                scalar_experts>0 offloads the last N experts' scaled copies
                to ScalarE (activation Copy with per-partition scale) --
                used for samples 0/1 where mix latency gates the conv and
                ScalarE is otherwise idle.
                """
                kt = kt_pool.tile([128, KWID], bf16, name="kt", tag="kt")
                ts = [t_pool.tile([128, KWID], bf16, name=f"t{e}",
                                  tag=f"t{e}") for e in range(E)]
                u = [u_pool.tile([128, KWID], bf16, name=f"u{i}", tag=f"u{i}")
                     for i in range(6)]
                r = lambda e: scal_sb[:, b * E + e:b * E + e + 1]
                for (a, z) in groups:
                    for e in range(E):
                        if e >= E - scalar_experts:
                            nc.scalar.activation(
                                out=ts[e][:, a:z],
                                in_=bank_sb[:, e * KWID + a:e * KWID + z],
                                func=Act.Copy, scale=r(e))
                        else:
                            nc.vector.tensor_scalar_mul(
                                out=ts[e][:, a:z],
                                in0=bank_sb[:, e * KWID + a:e * KWID + z],
                                scalar1=r(e))
                    pairs = [(u[0], ts[0], ts[1]), (u[1], ts[2], ts[3]),
                             (u[2], ts[4], ts[5]), (u[3], ts[6], ts[7]),
                             (u[4], u[0], u[1]), (u[5], u[2], u[3]),
                             (kt, u[4], u[5])]
                    for (o, i0, i1) in pairs:
                        nc.vector.tensor_add(out=o[:, a:z], in0=i0[:, a:z],
                                             in1=i1[:, a:z])
                return kt

            def emit_conv(b, xp, kt, dense, last):
                """Conv for sample b: accumulate 2c*9shift into 7 PSUM tiles
                from the 8-buffer rotating pool.

                dense=True (samples 1+): tiles 0,2,4 run all 18 rounds first
                (stopping 10-20us before sample end, DVE-drained at leisure),
                then tiles 1,3,5,6 round-major with ScalarE drains at sample
                end. With the +7 bank rotation, every drain the NEXT sample
                needs within ~7us comes from a dense tile that stopped long
                ago, so PSUM recycling never stalls the PE regardless of how
                the Tile scheduler orders drains among mix work.

                dense=False (sample 0): plain round-major, all drains on
                ScalarE -- required because sample 0's kt is mixed in column
                groups and round-major consumes kt shift by shift."""
                cps = [cps_pool.tile([128, NFREE], f32, name="cps", tag="cps")
                       for _ in range(NTILES)]
                x3 = [xp[c].rearrange("p (r q) -> p r q", q=WP)
                      for c in range(CHUNKS)]
                o = out_pool.tile([128, H * W], bf16, name="osb", tag="osb")

                def mm(n, c, s, first, stop):
                    dh, dw = s // KK, s % KK
                    lhsT = kt[:, c * KCOLS + s * 128:c * KCOLS + (s + 1) * 128]
                    rhs = x3[c][:, n * ROWS_PER_TILE + dh:
                                n * ROWS_PER_TILE + dh + ROWS_PER_TILE,
                                dw:dw + W]
                    nc.tensor.matmul(cps[n][:], lhsT, rhs,
                                     start=first, stop=stop)

                def drain(n, on_dve):
                    dst = o[:, n * NFREE:(n + 1) * NFREE]
                    if on_dve:
                        nc.vector.tensor_copy(out=dst, in_=cps[n][:])
                    else:
                        nc.scalar.activation(out=dst, in_=cps[n][:],
                                             func=Act.Copy)
                    if last:
                        # per-tile stores so the epilogue tail is one small
                        # DMA behind the final drain, not one big transfer;
                        # issue on both HWDGE rings so the ~0.6us issue slots
                        # don't serialize on one queue
                        eng = nc.sync if n % 2 == 0 else nc.scalar
                        eng.dma_start(
                            out=out_d[b, :,
                                      n * ROWS_PER_TILE:(n + 1) * ROWS_PER_TILE,
                                      :],
                            in_=o[:, n * NFREE:(n + 1) * NFREE])

                rounds = [(c, s) for c in range(CHUNKS) for s in range(NSH)]
                if dense == "semi":
                    # sample 1: kt arrives chunk-by-chunk barely ahead of
                    # the conv, so consume chunk0 as long as possible (t0's
                    # chunk0 rounds, then the late tiles' chunk0 rounds)
                    # before any chunk1 matmul; PSUM needs stay covered by
                    # sample 0's ScalarE drains.
                    late = (1, 3, 5, 6)
                    for i, (c, s) in enumerate(rounds[:NSH]):
                        mm(0, c, s, first=(i == 0), stop=False)
                    for i, (c, s) in enumerate(rounds[:NSH]):
                        for n in late:
                            mm(n, c, s, first=(i == 0), stop=False)
                    for j, (c, s) in enumerate(rounds[NSH:]):
                        mm(0, c, s, first=False, stop=(j == NSH - 1))
                    drain(0, on_dve=True)
                    for j, (c, s) in enumerate(rounds[NSH:]):
                        for n in late:
                            mm(n, c, s, first=False, stop=(j == NSH - 1))
                            if j == NSH - 1:
                                drain(n, on_dve=False)
                    for n in (2, 4):
                        for i, (c, s) in enumerate(rounds):
                            mm(n, c, s, first=(i == 0), stop=(i == 17))
                        drain(n, on_dve=True)
                    nc.sync.dma_start(out=out_d[b], in_=o[:])
                    return
                if dense:
                    for n in (0, 2, 4):
                        for i, (c, s) in enumerate(rounds):
                            mm(n, c, s, first=(i == 0), stop=(i == 17))
                        drain(n, on_dve=True)
                    late = (1, 3, 5, 6)
                else:
                    late = tuple(range(NTILES))
                for i, (c, s) in enumerate(rounds):
                    for k, n in enumerate(late):
                        mm(n, c, s, first=(i == 0), stop=(i == 17))
                        if i == 17:
                            # last sample: alternate engines so the final
                            # drains finish ~2x sooner
                            drain(n, on_dve=(last and k % 2 == 1))
                if not last:
                    nc.sync.dma_start(out=out_d[b], in_=o[:])

            # ---- software-pipelined emission ------------------------------
            # Sync-queue FIFO order gives DMA priority: watbat, xp(0), bank,
            # xp(1), xp(2), then per-iteration xp prefetch 3 samples ahead
            # behind each sample's output store. routing(b) is emitted with
            # gap(b) -- a full iteration before mix(b) -- so on the DVE queue
            # mix(b+2) sits right behind conv(b)'s ping-pong drains with its
            # sigmoid dependency long resolved, and the drains run the moment
            # conv(b) ends (PSUM recycling never waits on mix work).
            xps, gqs, kts = {}, {}, {}
            xps[0] = emit_xp(0)
            gqs[0] = emit_gap(0, xps[0])
            emit_bank_dma()
            emit_routing(0, gqs[0])
            kts[0] = emit_mix(0, [(0, 384), (384, 1152), (1152, KWID)])
            xps[1] = emit_xp(1)
            gqs[1] = emit_gap(1, xps[1])
            emit_routing(1, gqs[1])
            kts[1] = emit_mix(1, [(0, 1152), (1152, KWID)])
            xps[2] = emit_xp(2)
            gqs[2] = emit_gap(2, xps[2])
            emit_routing(2, gqs[2])
            for b in range(S):
                emit_conv(b, xps.pop(b), kts.pop(b), dense=(b > 0),
                          last=(b == S - 1))
                if b + 2 < S:
                    kts[b + 2] = emit_mix(b + 2, [(0, KWID)])
                if b + 3 < S:
                    xps[b + 3] = emit_xp(b + 3)
                    gqs[b + 3] = emit_gap(b + 3, xps[b + 3])
                    emit_routing(b + 3, gqs.pop(b + 3))

    nc.compile()
    return nc


def _prep_core_inputs(x, convs, W_att, b_att):
    """Host-side shard/layout prep. Returns list of 8 per-core input dicts."""
    import ml_dtypes
    f32 = np.float32
    bf16 = ml_dtypes.bfloat16
    # padded input, cin split into 2 chunks of 128
    xpad = np.zeros((B, CHUNKS, 128, HP, WP), dtype=bf16)
    xpad[:, :, :, 1:H + 1, 1:W + 1] = np.ascontiguousarray(x, dtype=f32).reshape(
        B, CHUNKS, 128, H, W).astype(bf16)
    xpad = xpad.reshape(B, CHUNKS, 128, PHW)

    # bank[half][c, g, p, e, kw*128 + m] = convs[e, half*128+m, c*128+p, g, kw]
    cv = np.ascontiguousarray(convs, dtype=f32).reshape(
        E, 2, MHALF, CHUNKS, 128, KK, KK)
    bank_halves = [
        np.ascontiguousarray(cv[:, h].transpose(2, 4, 3, 0, 5, 1).reshape(
            CHUNKS, 3, 128, E, 3 * 128)).astype(bf16)
        for h in range(2)
    ]

    watt = (np.asarray(W_att, dtype=f32).T / f32(H * W)).astype(f32)  # [CIN, E]
    watbat = np.empty((128, 3 * E), dtype=f32)
    watbat[:, 0:E] = watt[:128]
    watbat[:, E:2 * E] = watt[128:]
    # b_att/128 on every partition: the routing partition all-reduce sums it
    # back to b_att, so no separate bias add is needed
    watbat[:, 2 * E:3 * E] = np.broadcast_to(
        np.asarray(b_att, dtype=f32) / f32(128), (128, E))

    in_maps = []
    for k in range(NCORES):
        pair, half = k // 2, k % 2
        sl = slice(pair * SAMPLES_PER_CORE, (pair + 1) * SAMPLES_PER_CORE)
        in_maps.append({
            "xpad": np.ascontiguousarray(xpad[sl]),
            "bank": bank_halves[half],
            "watbat": watbat,
        })
    return in_maps


def _assemble_output(results):
    out = np.empty((B, COUT, H, W), dtype=np.float32)
    for k in range(NCORES):
        pair, half = k // 2, k % 2
        sl = slice(pair * SAMPLES_PER_CORE, (pair + 1) * SAMPLES_PER_CORE)
        out[sl, half * MHALF:(half + 1) * MHALF] = np.asarray(
            results[k]["out"], dtype=np.float32)
    return out


def kernel(x, convs, W_att, b_att):
    from concourse.bass_utils import run_bass_kernel_spmd

    if "nc" not in _cached:
        _cached["nc"] = _build_program()
    in_maps = _prep_core_inputs(x, convs, W_att, b_att)
    res = run_bass_kernel_spmd(_cached["nc"], in_maps, core_ids=list(range(NCORES)))
    return _assemble_output(res.results)
